# revision 14
# baseline (speedup 1.0000x reference)
"""DeepSeekMoE layer on 8 Trainium2 NeuronCores — sparse expert dispatch.

Problem (hardcoded): B=2, T=1024, C=1024, H=4096, E_routed=8 (top-2 sigmoid
gating), E_shared=2, fp32 reference; rel-L2 tolerance 2e-2.

Key idea vs the dense baseline: only compute the top-2 routed experts per
token (4096 token-expert passes globally instead of 16384), and run all
matmuls in bf16 (full PE rate at any moving width; halves weight DMA).

Sharding: 4-way expert-parallel x 2-way token-parallel.
  Core c (g = c//2 expert group, k = c%2 token half) owns routed experts
  {2g, 2g+1} gathered from token half k (1024 "pool" tokens), plus shared
  expert (g%2) on a 512-token quarter of its pool. The pool token order is
  permuted per-core so the shared quarter always occupies pool slots 0..511
  (keeps the program SPMD-uniform). Host sums the 4 per-half partials plus
  the residual.

Routing/gating and rmsnorm run on the host (tiny: 2048x8 gating, one
normalize); the host also builds one-hot gather matrices P [pool->cap] and
gate-weighted scatter matrices S [cap->pool] per (core, expert). On device,
gather/scatter are PE matmuls; the routed MLP2 output (C-major) is PE-
transposed to token-major so the scatter can contract over the capacity dim.

Device per-core program:
  shared MLP1 (xnT C-major moving, W1 stationary) -> gelu -> h_s
  shared MLP2 (h_s moving, W2 stationary, b2 seeded via K=1 matmul) -> y_cm
    -> PE transpose -> y_tok_s (token-major)
  per routed expert e: gather xg = XN_tok^T P_e -> MLP1 -> gelu -> MLP2
    -> y_cm -> PE transpose -> y_tok_r[e]
  output stripes [128 tok, 512 C]: identity-scatter of y_tok_s (quarter)
    + gate-scatter S_e^T y_tok_r[e] accumulated in PSUM -> evict -> DMA.

loop_m > 1 wraps the body in a hardware For_i loop (timing only).
"""
import contextlib
import os
import numpy as np
import ml_dtypes

import concourse.bass as bass
import concourse.tile as tile
from concourse.alu_op_type import AluOpType
from concourse import bacc, mybir
from concourse import bass_utils
from concourse.masks import make_identity

F32 = mybir.dt.float32
BF = mybir.dt.bfloat16
AF = mybir.ActivationFunctionType
BF_NP = ml_dtypes.bfloat16

B, T, C, H = 2, 1024, 1024, 4096
E_R, E_S, TOPK = 8, 2, 2
NTOK = B * T            # 2048
NCORES = 8
NPOOL = 1024            # tokens per core pool (half)
NQ = 512                # shared-quarter tokens per core
CK = C // 128           # 8
HK = H // 128           # 32
NBLK = HK // 4          # 8 w1/w2 stream blocks (4 h-chunks each)
EPS = 1.1920929e-07

_CACHE = {}
_STATE = {"cap": None}


# --------------------------------------------------------------------------
# device program
# --------------------------------------------------------------------------

def _build_program(loop_m=1, cap=None):
    if cap is None:
        cap = _STATE["cap"]
    assert cap is not None, "_prep_inputs must run before _build_program"
    jch = (cap + 127) // 128
    capp = 128 * jch

    nc = bacc.Bacc("TRN2", target_bir_lowering=False, debug=False,
                   enable_asserts=False)

    d = {}
    d["xn_tok"] = nc.dram_tensor("xn_tok", [128, 8 * 1024], BF, kind="ExternalInput").ap()
    d["xnT"] = nc.dram_tensor("xnT", [128, CK * NQ], BF, kind="ExternalInput").ap()
    d["w1s"] = nc.dram_tensor("w1s", [NBLK, 128, 4096], BF, kind="ExternalInput").ap()
    d["w2s"] = nc.dram_tensor("w2s", [NBLK, 128, 4096], BF, kind="ExternalInput").ap()
    d["w1r"] = nc.dram_tensor("w1r", [2, NBLK, 128, 4096], BF, kind="ExternalInput").ap()
    d["w2r"] = nc.dram_tensor("w2r", [2, NBLK, 128, 4096], BF, kind="ExternalInput").ap()
    d["pg"] = nc.dram_tensor("pg", [2, 128, 8 * cap], BF, kind="ExternalInput").ap()
    d["sg"] = nc.dram_tensor("sg", [2, 128, jch * 1024], BF, kind="ExternalInput").ap()
    d["b1"] = nc.dram_tensor("b1", [128, 3 * HK], F32, kind="ExternalInput").ap()
    d["b2r"] = nc.dram_tensor("b2r", [128, 2 * CK], F32, kind="ExternalInput").ap()
    d["b2s"] = nc.dram_tensor("b2s", [1, C], BF, kind="ExternalInput").ap()
    d["out"] = nc.dram_tensor("outT", [128, 8 * 1024], BF, kind="ExternalOutput").ap()

    with tile.TileContext(nc) as tc:
        with (
            tc.tile_pool(name="cst", bufs=1) as cst,
            tc.tile_pool(name="io", bufs=1) as io,
            tc.tile_pool(name="hs", bufs=1) as hs,
            tc.tile_pool(name="hr", bufs=1) as hr,
            tc.tile_pool(name="yp", bufs=2) as yp,
            tc.tile_pool(name="ytr", bufs=1) as ytr,
            tc.tile_pool(name="w1p", bufs=2) as w1p,
            tc.tile_pool(name="w2p", bufs=2) as w2p,
            tc.tile_pool(name="ot", bufs=3) as ot,
            tc.tile_pool(name="pp", bufs=8, space="PSUM") as pp,
        ):
            pools = dict(cst=cst, io=io, hs=hs, hr=hr, yp=yp, ytr=ytr,
                         w1p=w1p, w2p=w2p, ot=ot, pp=pp)
            loop = tc.For_i(0, loop_m, 1) if loop_m > 1 else contextlib.nullcontext()
            with loop:
                _moe_body(nc, d, pools, cap, jch, capp)

    nc.compile()
    return nc


def _moe_body(nc, d, p, cap, jch, capp):
    cst, io, hs, hr, yp, ytr, w1p, w2p, ot, pp = (
        p["cst"], p["io"], p["hs"], p["hr"], p["yp"], p["ytr"],
        p["w1p"], p["w2p"], p["ot"], p["pp"])

    # ---- constants ----
    identf = cst.tile([128, 128], F32, tag="identf", name="identf")
    make_identity(nc, identf[:])
    ident = cst.tile([128, 128], BF, tag="ident", name="ident")
    nc.vector.tensor_copy(ident[:], identf[:])
    ones_f = cst.tile([1, 512], F32, tag="ones_f", name="ones_f")
    nc.gpsimd.memset(ones_f[:], 1.0)
    onesr = cst.tile([1, 128], BF, tag="onesr", name="onesr")
    nc.vector.tensor_copy(onesr[:], ones_f[:, :128])

    # ---- input loads ----
    xn_tok = io.tile([128, 8 * 1024], BF, tag="xn_tok", name="xn_tok")
    nc.sync.dma_start(xn_tok[:, :4096], d["xn_tok"][:, :4096])
    nc.sync.dma_start(xn_tok[:, 4096:], d["xn_tok"][:, 4096:])
    xnT = io.tile([128, CK * NQ], BF, tag="xnT", name="xnT")
    nc.sync.dma_start(xnT[:], d["xnT"])
    pg = []
    for e in range(2):
        pge = io.tile([128, 8 * cap], BF, tag=f"pg{e}", name=f"pg{e}")
        nc.sync.dma_start(pge[:], d["pg"][e])
        pg.append(pge)
    sg = []
    for e in range(2):
        sge = io.tile([128, jch * 1024], BF, tag=f"sg{e}", name=f"sg{e}")
        nc.sync.dma_start(sge[:], d["sg"][e])
        sg.append(sge)
    b1 = io.tile([128, 3 * HK], F32, tag="b1", name="b1")
    nc.sync.dma_start(b1[:], d["b1"])
    b2r = io.tile([128, 2 * CK], F32, tag="b2r", name="b2r")
    nc.sync.dma_start(b2r[:], d["b2r"])
    b2s = io.tile([1, C], BF, tag="b2s", name="b2s")
    nc.sync.dma_start(b2s[:], d["b2s"])

    # ---- shared expert MLP1: h_s[hh] = gelu(W1s^T xnT + b1) ----
    h_s = []
    for blk in range(NBLK):
        w1t = w1p.tile([128, 4096], BF, tag="w1blk", name=f"w1s_{blk}")
        nc.sync.dma_start(w1t[:], d["w1s"][blk])
        for i in range(4):
            hh = 4 * blk + i
            ph = pp.tile([128, 512], F32, tag="pp", name=f"phs_{hh}")
            for k in range(CK):
                nc.tensor.matmul(
                    ph[:], w1t[:, 1024 * i + 128 * k:1024 * i + 128 * (k + 1)],
                    xnT[:, NQ * k:NQ * (k + 1)],
                    start=(k == 0), stop=(k == CK - 1))
            ht = hs.tile([128, 512], BF, tag=f"hs{hh}", name=f"hs_{hh}")
            nc.scalar.activation(ht[:], ph[:], AF.Gelu, bias=b1[:, hh:hh + 1])
            h_s.append(ht)

    # ---- shared expert MLP2, token-major direct: y[t, c] = h_s^T W2s + b2s ----
    pys = [pp.tile([128, 512], F32, tag="pp", name=f"pys_{t}_{hf}")
           for t in range(4) for hf in range(2)]
    for t in range(4):
        for hf in range(2):
            nc.tensor.matmul(pys[2 * t + hf][:], onesr[:],
                             b2s[:, 512 * hf:512 * (hf + 1)], start=True, stop=False)
    for blk in range(NBLK):
        w2t = w2p.tile([128, 4096], BF, tag="w2blk", name=f"w2s_{blk}")
        nc.sync.dma_start(w2t[:], d["w2s"][blk])
        for i in range(4):
            hh = 4 * blk + i
            for t in range(4):
                for hf in range(2):
                    nc.tensor.matmul(
                        pys[2 * t + hf][:], h_s[hh][:, 128 * t:128 * (t + 1)],
                        w2t[:, 1024 * i + 512 * hf:1024 * i + 512 * (hf + 1)],
                        start=False, stop=(hh == HK - 1))
    y_tok_s = []
    for t in range(4):
        yts = hs.tile([128, 1024], BF, tag=f"ytoks{t}", name=f"ytoks{t}")
        for hf in range(2):
            nc.vector.tensor_copy(yts[:, 512 * hf:512 * (hf + 1)], pys[2 * t + hf][:])
        y_tok_s.append(yts)

    # ---- routed experts ----
    y_tok_r = [[None] * jch for _ in range(2)]
    for e in range(2):
        # gather: xg[k] = xn_tok^T P_e  (C-major, cap tokens)
        xg = yp.tile([128, CK * cap], BF, tag="xg", name=f"xg{e}")
        for k in range(CK):
            gps = pp.tile([128, cap], F32, tag="pp", name=f"gps_{e}_{k}")
            for tch in range(8):
                nc.tensor.matmul(
                    gps[:], xn_tok[:, 1024 * tch + 128 * k:1024 * tch + 128 * (k + 1)],
                    pg[e][:, cap * tch:cap * (tch + 1)],
                    start=(tch == 0), stop=(tch == 7))
            nc.scalar.copy(xg[:, cap * k:cap * (k + 1)], gps[:])
        # MLP1
        h_r = []
        for blk in range(NBLK):
            w1t = w1p.tile([128, 4096], BF, tag="w1blk", name=f"w1r_{e}_{blk}")
            nc.sync.dma_start(w1t[:], d["w1r"][e, blk])
            for i in range(4):
                hh = 4 * blk + i
                ph = pp.tile([128, cap], F32, tag="pp", name=f"phr_{e}_{hh}")
                for k in range(CK):
                    nc.tensor.matmul(
                        ph[:], w1t[:, 1024 * i + 128 * k:1024 * i + 128 * (k + 1)],
                        xg[:, cap * k:cap * (k + 1)],
                        start=(k == 0), stop=(k == CK - 1))
                ht = hr.tile([128, cap], BF, tag=f"hr{hh}", name=f"hr_{e}_{hh}")
                nc.scalar.activation(ht[:], ph[:], AF.Gelu,
                                     bias=b1[:, HK * (1 + e) + hh:HK * (1 + e) + hh + 1])
                h_r.append(ht)
        # MLP2 (C-major); b2 is added during eviction
        pyr = [pp.tile([128, cap], F32, tag="pp", name=f"pyr_{e}_{k}")
               for k in range(CK)]
        for blk in range(NBLK):
            w2t = w2p.tile([128, 4096], BF, tag="w2blk", name=f"w2r_{e}_{blk}")
            nc.sync.dma_start(w2t[:], d["w2r"][e, blk])
            for i in range(4):
                hh = 4 * blk + i
                for k in range(CK):
                    nc.tensor.matmul(
                        pyr[k][:], w2t[:, 1024 * i + 128 * k:1024 * i + 128 * (k + 1)],
                        h_r[hh][:], start=(hh == 0), stop=(hh == HK - 1))
        # evict with fused b2 add (zero-padded to capp), transpose to token-major
        for k in range(CK):
            yc = yp.tile([128, capp], BF, tag=f"yrcm{k}", name=f"yrcm_{e}_{k}")
            nc.vector.tensor_scalar(yc[:, :cap], pyr[k][:],
                                    b2r[:, CK * e + k:CK * e + k + 1], None,
                                    AluOpType.add)
            if capp > cap:
                nc.gpsimd.memset(yc[:, cap:], 0.0)
            if k == 0:
                yr_cm = []
            yr_cm.append(yc)
        for j in range(jch):
            ytp = pp.tile([128, 1024], BF, tag="pp", name=f"ytpr_{e}_{j}")
            for k in range(CK):
                nc.tensor.transpose(ytp[:, 128 * k:128 * (k + 1)],
                                    yr_cm[k][:, 128 * j:128 * (j + 1)], ident[:])
            ytt = ytr.tile([128, 1024], BF, tag=f"ytr{e}_{j}", name=f"ytr_{e}_{j}")
            nc.vector.tensor_copy(ytt[:], ytp[:])
            y_tok_r[e][j] = ytt

    # ---- output stripes: residual-less partial = shared + routed scatters ----
    for tch in range(8):
        for half in range(2):
            acc = pp.tile([128, 512], F32, tag="pp", name=f"acc_{tch}_{half}")
            ops = []
            if tch < 4:
                ops.append(("sh",))
            for e in range(2):
                for j in range(jch):
                    ops.append(("rt", e, j))
            for idx, op in enumerate(ops):
                st, sp = (idx == 0), (idx == len(ops) - 1)
                if op[0] == "sh":
                    nc.tensor.matmul(acc[:], ident[:],
                                     y_tok_s[tch][:, 512 * half:512 * (half + 1)],
                                     start=st, stop=sp)
                else:
                    _, e, j = op
                    nc.tensor.matmul(
                        acc[:], sg[e][:, 1024 * j + 128 * tch:1024 * j + 128 * (tch + 1)],
                        y_tok_r[e][j][:, 512 * half:512 * (half + 1)],
                        start=st, stop=sp)
            outt = ot.tile([128, 512], BF, tag="out", name=f"out_{tch}_{half}")
            nc.scalar.copy(outt[:], acc[:])
            nc.sync.dma_start(d["out"][:, 1024 * tch + 512 * half:1024 * tch + 512 * (half + 1)],
                              outt[:])


# --------------------------------------------------------------------------
# host prep
# --------------------------------------------------------------------------

def _routing(u2, centroids):
    """scores/top-2/normalized gates, matching the jax reference."""
    f = np.float32
    scores = 1.0 / (1.0 + np.exp(-(u2 @ np.asarray(centroids, f))))
    top2 = np.argsort(-scores, axis=1, kind="stable")[:, :TOPK]
    denom = scores.sum(axis=1, keepdims=True)
    gk = np.take_along_axis(scores, top2, axis=1) / denom
    gmat = np.zeros((NTOK, E_R), f)
    np.put_along_axis(gmat, top2, gk.astype(f), axis=1)
    return top2, gmat


def _pack_w1(w1):
    # [C, H] -> [NBLK, 128, 4096]: [blk, p, 1024*i + 128*k + j] = w1[128k+p, 128(4blk+i)+j]
    a = w1.reshape(CK, 128, NBLK, 4, 128).transpose(2, 1, 3, 0, 4)
    return np.ascontiguousarray(a).reshape(NBLK, 128, 4096)


def _pack_w2(w2):
    # [H, C] -> [NBLK, 128, 4096]: [blk, p, 1024*i + c] = w2[128(4blk+i)+p, c]
    a = w2.reshape(NBLK, 4, 128, C).transpose(0, 2, 1, 3)
    return np.ascontiguousarray(a).reshape(NBLK, 128, 4096)


def _prep_inputs(u, g_shared, W1_s, b1_s, W2_s, b2_s,
                 g_routed, W1_r, b1_r, W2_r, b2_r, centroids):
    f = np.float32
    u2 = np.ascontiguousarray(np.asarray(u, f).reshape(NTOK, C))
    rms = np.sqrt(np.mean(u2 * u2, axis=1, keepdims=True) + EPS)
    xn = (u2 / rms)
    gsh = np.asarray(g_shared, f).reshape(C, 1)
    grt = np.asarray(g_routed, f).reshape(C, 1)
    top2, gmat = _routing(u2, centroids)

    # per-(core, local expert) selections, global capacity
    sels = {}
    maxc = 1
    for c in range(NCORES):
        g, k = c // 2, c % 2
        hq = g // 2
        q_sh = 2 * k + hq
        q_ot = 2 * k + (1 - hq)
        pool_idx = np.concatenate([
            np.arange(NQ * q_sh, NQ * (q_sh + 1)),
            np.arange(NQ * q_ot, NQ * (q_ot + 1))])
        t2p = top2[pool_idx]
        for el in range(2):
            eg = 2 * g + el
            sel = np.nonzero((t2p == eg).any(axis=1))[0]
            sels[(c, el)] = (pool_idx, sel)
            maxc = max(maxc, len(sel))
    cap = ((maxc + 7) // 8) * 8
    assert cap <= 512, f"routed capacity {maxc} too imbalanced"
    jch = (cap + 127) // 128
    _STATE["cap"] = cap

    in_maps = []
    aux_pool = []
    group_cache = {}
    for c in range(NCORES):
        g, k = c // 2, c % 2
        s = g % 2
        if g not in group_cache:
            w1r = np.stack([_pack_w1((grt * np.asarray(W1_r[2 * g + el], f)))
                            for el in range(2)]).astype(BF_NP)
            w2r = np.stack([_pack_w2(np.asarray(W2_r[2 * g + el], f))
                            for el in range(2)]).astype(BF_NP)
            w1s = _pack_w1(gsh * np.asarray(W1_s[s], f)).astype(BF_NP)
            w2s = _pack_w2(np.asarray(W2_s[s], f)).astype(BF_NP)
            b1c = np.stack([np.asarray(b1_s[s], f)] +
                           [np.asarray(b1_r[2 * g + el], f) for el in range(2)])
            b1t = np.ascontiguousarray(
                b1c.reshape(3, HK, 128).transpose(2, 0, 1)).reshape(128, 3 * HK)
            # [128, 2*CK] f32: col CK*e + k holds b2_r[2g+e][128k:128(k+1)]
            b2rw = np.ascontiguousarray(
                np.stack([np.asarray(b2_r[2 * g + el], f) for el in range(2)])
                .reshape(2, CK, 128).transpose(2, 0, 1)).reshape(128, 2 * CK)
            b2sw = np.asarray(b2_s[s], f).reshape(1, C).astype(BF_NP)
            group_cache[g] = (w1r, w2r, w1s, w2s, b1t, b2rw, b2sw)
        w1r, w2r, w1s, w2s, b1t, b2rw, b2sw = group_cache[g]

        pool_idx, _ = sels[(c, 0)]
        xnp = xn[pool_idx]                                   # [1024, C] f32
        xn_tok = np.ascontiguousarray(
            xnp.reshape(8, 128, C).transpose(1, 0, 2)).reshape(128, 8 * C).astype(BF_NP)
        xnT = np.ascontiguousarray(
            xnp[:NQ].T.reshape(CK, 128, NQ).transpose(1, 0, 2)).reshape(128, CK * NQ).astype(BF_NP)

        pmat = np.zeros((2, NPOOL, cap), f)
        smat = np.zeros((2, jch * 128, NPOOL), f)
        for el in range(2):
            eg = 2 * g + el
            _, sel = sels[(c, el)]
            n = len(sel)
            pmat[el, sel, np.arange(n)] = 1.0
            smat[el, np.arange(n), sel] = gmat[pool_idx[sel], eg]
        pgm = np.ascontiguousarray(
            pmat.reshape(2, 8, 128, cap).transpose(0, 2, 1, 3)).reshape(2, 128, 8 * cap).astype(BF_NP)
        sgm = np.ascontiguousarray(
            smat.reshape(2, jch, 128, NPOOL).transpose(0, 2, 1, 3)).reshape(2, 128, jch * NPOOL).astype(BF_NP)

        in_maps.append({
            "xn_tok": xn_tok, "xnT": xnT,
            "w1s": w1s, "w2s": w2s, "w1r": w1r, "w2r": w2r,
            "pg": pgm, "sg": sgm,
            "b1": b1t, "b2r": b2rw, "b2s": b2sw,
        })
        aux_pool.append(pool_idx)
    return in_maps, (u2, aux_pool)


def _run(in_maps, trace=False):
    cap = _STATE["cap"]
    key = ("nc", cap)
    if key not in _CACHE:
        _CACHE[key] = _build_program(cap=cap)
    nc = _CACHE[key]
    res = bass_utils.run_bass_kernel_spmd(
        nc, in_maps, core_ids=list(range(NCORES)), trace=trace)
    return res


def kernel(**inputs):
    in_maps, (u2, aux_pool) = _prep_inputs(**inputs)
    trace = bool(int(os.environ.get("MOE_TRACE", "0")))
    res = _run(in_maps, trace=trace)
    _CACHE["last_results"] = res
    out2 = u2.astype(np.float64)
    for c in range(NCORES):
        part = (res.results[c]["outT"].astype(np.float64)
                .reshape(128, 8, 1024).transpose(1, 0, 2).reshape(NPOOL, C))
        out2[aux_pool[c]] += part
    return out2.astype(np.float32).reshape(B, T, C)
